# revision 70
# baseline (speedup 1.0000x reference)
"""Trainium2 Bass kernel for the HNX scatter-memory block.

Sharding: 8 cores = (batch b in 0..3) x (sequence half j in 0..1).
Each core processes its 1024-token window plus W warmup tokens on each
side (zero-padded + masked at sequence edges), so both the forward and
backward EMA scans converge to near the exact state before the window
starts (truncation error ~ sigmoid(decay)^W ~ 2e-3 at W=32).  No
inter-core communication.

On-chip layout is "scan layout": channels on partitions, time along the
free dimension.  Key optimizations over a straightforward port:

- All matmul operands and elementwise intermediates are bf16 (fp32 PSUM
  accumulation, fp32 per-channel scalars): 1 cycle/row on the PE and
  the 2x/4x DVE perf modes.
- The conv center tap k1 is folded into the in-proj weights on the
  host, so the causal conv is y_t = (k0/k1)*x1_{t-1} + x1_t (one 4x
  tensor_scalar + one gpsimd add).
- The memory-read contribution is folded into the out-projection via
  Wmo = mem_bank @ W_out (host-precomputed): W_out^T(memb^T E2) =
  Wmo^T E2 joins the out-proj PSUM accumulation chains directly.
- Phase-1 elementwise runs per 512-token chunk, pipelined with the
  in-proj matmul chains; the entropy/slot/mem reduction chains and the
  second window chunk's work are emission-ordered to keep the PE busy
  through the gate/fusion tail.
- ACT activation-table thrash is avoided by ordering all exps after
  the last phase-1 silu (explicit dep) and keeping the gate sigmoids
  in exp/reciprocal form (same ACT table as Ln/Exp/Identity).
- DMA issue costs ~650ns of sequencer time per dma_start, so inputs
  are consolidated (one packed param tensor, one xt tile, two-group
  weight loads) and issued from otherwise-idle engine queues.
"""

import numpy as np
from contextlib import ExitStack

import ml_dtypes
import concourse.bacc as bacc
import concourse.tile as tile
from concourse import mybir
from concourse.bass_utils import run_bass_kernel_spmd

F32 = mybir.dt.float32
BF16 = mybir.dt.bfloat16
AF = mybir.ActivationFunctionType
OP = mybir.AluOpType


class Cfg:
    def __init__(self, DI=1024, H=1024, O=1024, S=128, T=2048, W=64, CH=512,
                 sim_acts=False):
        self.DI, self.H, self.O, self.S, self.T, self.W, self.CH = DI, H, O, S, T, W, CH
        self.Tout = T // 2            # tokens per core window
        self.Tw = self.Tout + 2 * W   # work tokens per core
        self.KG = DI // 128           # input k-tiles
        self.HG = H // 128            # hidden channel groups
        self.OG = O // 128            # output channel groups
        self.NCH = self.Tw // CH      # phase-1 chunks
        self.WCH = self.Tout // CH    # phase-3 (window) chunks
        self.sim_acts = sim_acts
        assert self.S == 128 and self.Tout % CH == 0
        assert CH <= 512 and self.W <= CH


# chp column layout: per-channel params, one column per (param, group)
CHP_NAMES = ["kr", "df", "db", "omdb", "omdf", "sbias", "ba"]
SC_F1, SC_F0, SC_NF1, SC_F2, SC_NSW, SC_NSB, SC_NBMG = range(7)


def build_program(cfg: Cfg):
    c = cfg
    nc = bacc.Bacc("TRN2", target_bir_lowering=False, debug=False,
                   enable_asserts=False)

    NPRM = len(CHP_NAMES) * c.HG + c.OG + 1
    xt = nc.dram_tensor("xt", [c.DI, c.Tw], BF16, kind="ExternalInput").ap()
    w_in = nc.dram_tensor("w_in", [c.DI, 2 * c.H], BF16, kind="ExternalInput").ap()
    w_out = nc.dram_tensor("w_out", [c.H, c.O], BF16, kind="ExternalInput").ap()
    w_slot = nc.dram_tensor("w_slot", [c.H, c.S], BF16, kind="ExternalInput").ap()
    w_mg = nc.dram_tensor("w_mg", [c.H, 1], BF16, kind="ExternalInput").ap()
    wmo_d = nc.dram_tensor("wmo", [c.S, c.O], BF16, kind="ExternalInput").ap()
    prm = nc.dram_tensor("prm", [128, NPRM], F32, kind="ExternalInput").ap()
    sc = nc.dram_tensor("sc", [1, 8], F32, kind="ExternalInput").ap()
    mask_d = nc.dram_tensor("mask", [1, c.Tw], BF16, kind="ExternalInput").ap()
    out_d = nc.dram_tensor("outT", [c.O, c.Tout], F32, kind="ExternalOutput").ap()

    with tile.TileContext(nc) as tc:
        with ExitStack() as top:
            consts = top.enter_context(tc.tile_pool(name="consts", bufs=1))
            # critical-path first: each dma_start costs ~650ns of issue time
            # on its issuing engine's sequencer, so the head transfers are
            # consolidated and spread across the idle engines.
            prm_t = consts.tile([128, NPRM], F32)
            nc.sync.dma_start(prm_t[:], prm[:])
            sc_t = consts.tile([1, 8], F32)
            nc.sync.dma_start(sc_t[:], sc[:])
            NCH0 = len(CHP_NAMES) * c.HG
            bout_t = prm_t[:, NCH0:NCH0 + c.OG]
            bslot_t = prm_t[:, NCH0 + c.OG:NCH0 + c.OG + 1]

            # x in scan layout, all k-tiles in one tile [128, k*Tw]
            xt_pool = top.enter_context(tc.tile_pool(name="xt", bufs=1))
            xta = xt_pool.tile([128, c.KG * c.Tw], BF16)
            xta_v = xta[:].rearrange("p (k t) -> p k t", t=c.Tw)
            xt_v = xt.rearrange("(k p) t -> p k t", p=128)
            KH = c.KG // 2
            nc.gpsimd.dma_start(xta_v[:, 0:KH, 0:c.CH], xt_v[:, 0:KH, 0:c.CH])
            nc.gpsimd.dma_start(xta_v[:, KH:c.KG, 0:c.CH],
                                xt_v[:, KH:c.KG, 0:c.CH])

            def xts(k, sl):
                return xta[:, k * c.Tw:(k + 1) * c.Tw][:, sl]

            win = top.enter_context(tc.tile_pool(name="win", bufs=2))

            WG = 2   # weight groups per DMA

            def load_w(g, eng_a=None, eng_d=None):
                """Load in-proj weights for groups [g, g+WG).  The pdt
                chain runs first, so wd transfers first."""
                gw = 128 * WG
                wd = win.tile([128, gw * c.KG], BF16, name="wd", tag="wd")
                (eng_d or nc.sync).dma_start(
                    wd[:].rearrange("p (k m) -> p k m", m=gw),
                    w_in[:, c.H + g * 128:c.H + g * 128 + gw]
                    .rearrange("(k p) m -> p k m", p=128))
                wa = win.tile([128, gw * c.KG], BF16, name="wa", tag="wa")
                (eng_a or nc.sync).dma_start(
                    wa[:].rearrange("p (k m) -> p k m", m=gw),
                    w_in[:, g * 128:g * 128 + gw]
                    .rearrange("(k p) m -> p k m", p=128))
                return wa, wd

            w_pre = load_w(0, eng_a=nc.scalar, eng_d=nc.scalar)
            ones_t = consts.tile([128, 1], BF16)
            nc.vector.memset(ones_t[:], 1.0)
            mb = consts.tile([128, c.Tw], BF16)
            nc.sync.dma_start(mb[:], mask_d.broadcast_to([128, c.Tw]))
            wsa = consts.tile([128, c.HG * c.S], BF16)
            nc.sync.dma_start(
                wsa[:].rearrange("p (k s) -> p k s", s=c.S),
                w_slot.rearrange("(k p) s -> p k s", p=128))
            wslot_t = [wsa[:, k * c.S:(k + 1) * c.S] for k in range(c.HG)]
            wmga = consts.tile([128, c.HG], BF16)
            nc.sync.dma_start(
                wmga[:].rearrange("p (k s) -> p k s", s=1),
                w_mg.rearrange("(k p) s -> p k s", p=128))
            wmg_t = [wmga[:, k:k + 1] for k in range(c.HG)]
            nc.sync.dma_start(xta_v[:, :, c.CH:c.Tw], xt_v[:, :, c.CH:c.Tw])
            wmo_t = consts.tile([128, c.O], BF16)
            # resident out-proj weights: [128, KG*OG*128] bf16, block (k,m)
            wo_t = consts.tile([128, c.HG * c.O], BF16)

            def chpc(name, g):
                i = CHP_NAMES.index(name) * c.HG + g
                return prm_t[:, i:i + 1]

            def scc(i):
                return sc_t[0:1, i:i + 1]

            fpool = top.enter_context(tc.tile_pool(name="f", bufs=1))
            f_t = [fpool.tile([128, c.Tw - c.W], BF16, name=f"f{g}", tag=f"f{g}")
                   for g in range(c.HG)]
            gpool = top.enter_context(tc.tile_pool(name="gb", bufs=1))
            gb_t = [gpool.tile([128, c.Tout], BF16, name=f"gb{g}", tag=f"gb{g}")
                    for g in range(c.HG)]

            pch = top.enter_context(tc.tile_pool(name="pch", bufs=3))
            scr = top.enter_context(tc.tile_pool(name="scr", bufs=2))
            p2s = top.enter_context(tc.tile_pool(name="p2s", bufs=2))
            p3 = top.enter_context(tc.tile_pool(name="p3", bufs=3))

            # phase-3 accumulators (PSUM): one bank per (pZ,pG,pM,pL),
            # single-buffered and reused across the two window chunks.
            pst = [None]   # tail-scope PSUM pool, created after phase 1
            pZ = [None] * c.WCH
            pG = [None] * c.WCH
            pM = [None] * c.WCH
            pL = [None] * c.WCH

            last_silu = [None]

            def red_fl(g, w):
                """Slot-logit + mem-gate chains: depend only on f_t[g], so
                chunk 0's run interleaved with phase 1 on the PE."""
                sl = slice(w * c.CH, (w + 1) * c.CH)
                if g == 0:
                    pM[w] = pst[0].tile([1, c.CH], F32, name=f"pM{w}",
                                        tag=f"pM{w}")
                    pL[w] = pst[0].tile([128, c.CH], F32, name=f"pL{w}",
                                        tag=f"pL{w}")
                st, sp = (g == 0), (g == c.HG - 1)
                nc.tensor.matmul(pM[w][:], wmg_t[g][:],
                                 f_t[g][:, sl], start=st, stop=sp)
                nc.tensor.matmul(pL[w][:], wslot_t[g][:],
                                 f_t[g][:, sl], start=st, stop=sp)

            def red_zg(g, w=None):
                """Entropy chains for BOTH window chunks (one full-window
                exp per group; with per-chunk accumulator banks the two
                chains advance in lockstep).  The exps are ordered after
                the last phase-1 silu (ACT-table batching)."""
                if g == 0:
                    for ww in range(c.WCH):
                        pZ[ww] = pst[0].tile([1, c.CH], F32, name=f"pZ{ww}",
                                             tag=f"pZ{ww}")
                        pG[ww] = pst[0].tile([1, c.CH], F32, name=f"pG{ww}",
                                             tag=f"pG{ww}")
                st, sp = (g == 0), (g == c.HG - 1)
                pt = p3.tile([128, c.Tout], BF16, name="pt", tag="p")
                pt_inst = nc.scalar.activation(pt[:], f_t[g][:, 0:c.Tout],
                                               AF.Exp)
                if last_silu[0] is not None:
                    tile.add_dep_helper(pt_inst.ins, last_silu[0].ins,
                                        sync=False,
                                        reason="act-table batching")
                for ww in range(c.WCH):
                    sl = slice(ww * c.CH, (ww + 1) * c.CH)
                    pft = p3.tile([128, c.CH], BF16, name="pft", tag="pf")
                    nc.vector.tensor_tensor(pft[:], f_t[g][:, sl],
                                            pt[:, sl], OP.mult)
                    nc.tensor.matmul(pZ[ww][:], ones_t[:], pt[:, sl],
                                     start=st, stop=sp)
                    nc.tensor.matmul(pG[ww][:], ones_t[:], pft[:],
                                     start=st, stop=sp)

            with ExitStack() as p1:
                ps1 = p1.enter_context(tc.tile_pool(name="ps1", bufs=2,
                                                    space="PSUM"))

                # phase-1 chunk edges (last chunk may be short)
                edges = list(range(0, c.Tw, c.CH)) + [c.Tw]

                for g in range(c.HG):
                    if g % WG == 0:
                        wa2, wd2 = w_pre
                        if g + WG < c.HG:
                            w_pre = load_w(g + WG)
                    gg = g % WG
                    gw = 128 * WG

                    def wslice(wt, k):
                        return wt[:, k * gw + gg * 128:k * gw + (gg + 1) * 128]

                    x1 = pch.tile([128, c.Tw], BF16, tag="x1")
                    tmp = pch.tile([128, c.Tw], BF16, tag="tc")
                    ypre = pch.tile([128, c.Tw], BF16, tag="tb")
                    ysl = pch.tile([128, c.Tw], BF16, tag="ysl")
                    u = pch.tile([128, c.Tw], BF16, tag="ta")
                    fscr = scr.tile([128, c.W], BF16, tag="fscr")
                    for n in range(len(edges) - 1):
                        sl = slice(edges[n], edges[n + 1])
                        cw = edges[n + 1] - edges[n]
                        pa = ps1.tile([128, cw], F32, tag="pa")
                        pdt = ps1.tile([128, cw], F32, tag="pdt")
                        for k in range(c.KG):
                            nc.tensor.matmul(
                                pdt[:], wslice(wd2, k),
                                xts(k, sl),
                                start=(k == 0), stop=(k == c.KG - 1))
                        for k in range(c.KG):
                            nc.tensor.matmul(
                                pa[:], wslice(wa2, k),
                                xts(k, sl),
                                start=(k == 0), stop=(k == c.KG - 1))
                        sdt = scr.tile([128, cw], BF16, tag="sdt")
                        if c.sim_acts:
                            # CoreSim has no Silu: silu(x) = x * sigmoid(x)
                            nc.scalar.activation(sdt[:], pdt[:], AF.Sigmoid,
                                                 bias=chpc("sbias", g))
                            nc.vector.scalar_tensor_tensor(
                                sdt[:], pdt[:], chpc("sbias", g), sdt[:],
                                OP.add, OP.mult)
                        else:
                            nc.scalar.activation(sdt[:], pdt[:], AF.Silu,
                                                 bias=chpc("sbias", g))
                        if n == 0 or n == len(edges) - 2:
                            # zero the pad region so the causal conv and
                            # scans see zeros outside the true sequence
                            nc.vector.tensor_tensor(sdt[:], sdt[:],
                                                    mb[:, sl], OP.mult)
                        # x1 = (k1*a + k1*b_a) * silu(dt + sbias)
                        # (k1 is folded into W_in / b_a on the host)
                        nc.vector.scalar_tensor_tensor(
                            x1[:, sl], pa[:], chpc("ba", g), sdt[:],
                            OP.add, OP.mult)

                    # per-chunk conv + silu + scan pipeline, in a second
                    # chunk loop so all the sdt acts precede the (slower)
                    # ysl chain on ACT — otherwise ysl head-of-line blocks
                    # the sdt that frees the next PSUM bank.
                    for n in range(len(edges) - 1):
                        s, e = edges[n], edges[n + 1]
                        if n == 0:
                            nc.gpsimd.memset(tmp[:, 0:1], 0.0)
                            nc.vector.tensor_scalar(
                                tmp[:, 1:e], x1[:, 0:e - 1],
                                chpc("kr", g), None, OP.mult)
                        else:
                            nc.vector.tensor_scalar(
                                tmp[:, s:e], x1[:, s - 1:e - 1],
                                chpc("kr", g), None, OP.mult)
                        if g >= c.HG - 2:
                            # last groups sit on the critical path into the
                            # reductions: use the faster engine
                            nc.vector.tensor_tensor(
                                ypre[:, s:e], tmp[:, s:e], x1[:, s:e], OP.add)
                        else:
                            nc.gpsimd.tensor_tensor(
                                ypre[:, s:e], tmp[:, s:e], x1[:, s:e], OP.add)
                        if c.sim_acts:
                            last_silu[0] = nc.scalar.activation(
                                ysl[:, s:e], ypre[:, s:e], AF.Sigmoid)
                            nc.vector.tensor_tensor(
                                ysl[:, s:e], ypre[:, s:e], ysl[:, s:e],
                                OP.mult)
                        else:
                            last_silu[0] = nc.scalar.activation(
                                ysl[:, s:e], ypre[:, s:e], AF.Silu)
                        nc.vector.tensor_scalar(u[:, s:e], ysl[:, s:e],
                                                chpc("omdf", g), None, OP.mult)

                        # fwd scan f = df*f + (1-df)*y, chained across
                        # chunks; first W tokens go to discard scratch
                        dfb = chpc("df", g)
                        if n == 0:
                            nc.vector.tensor_tensor_scan(
                                fscr[:], dfb.broadcast_to([128, c.W]),
                                u[:, 0:c.W], 0.0, OP.mult, OP.add)
                            nc.vector.tensor_tensor_scan(
                                f_t[g][:, 0:e - c.W],
                                dfb.broadcast_to([128, e - c.W]),
                                u[:, c.W:e], fscr[:, c.W - 1:c.W],
                                OP.mult, OP.add)
                        else:
                            nc.vector.tensor_tensor_scan(
                                f_t[g][:, s - c.W:e - c.W],
                                dfb.broadcast_to([128, e - s]),
                                u[:, s:e],
                                f_t[g][:, s - c.W - 1:s - c.W],
                                OP.mult, OP.add)

                # tail-only weights: emitted late so their DMAs don't
                # compete with the phase-1 critical path.
                nc.sync.dma_start(wmo_t[:], wmo_d[:])
                for k in range(c.HG):
                    nc.sync.dma_start(wo_t[:, k * c.O:(k + 1) * c.O],
                                      w_out[k * 128:(k + 1) * 128, :])


            # ------------- phase 3 tail: gates, memory, fusion, out ------
            with ExitStack() as p2:
                pb1 = p2.enter_context(tc.tile_pool(name="pb1", bufs=2))
                wpool = p2.enter_context(tc.tile_pool(name="wp", bufs=1))
                row = p2.enter_context(tc.tile_pool(name="row", bufs=1))
                # one bank per accumulator (both chunks fully
                # independent); the out-proj chains reuse four of these
                # banks by tag after the rows drain them.
                pst[0] = p2.enter_context(tc.tile_pool(name="pst", bufs=1,
                                                       space="PSUM"))

                # ---- phase 2: bwd scans (gb only feeds late fusion) ----
                for g in range(c.HG):
                    Lw = c.Tw - c.W
                    d1 = p2s.tile([128, Lw], BF16, name="d1", tag="d1")
                    nc.vector.tensor_scalar(d1[:], f_t[g][:], chpc("omdb", g),
                                            None, OP.mult)
                    # mask only matters in the right-pad = bwd warmup region
                    d1p = p2s.tile([128, c.W], BF16, name="d1p", tag="d1p")
                    nc.vector.tensor_tensor(d1p[:], d1[:, c.Tout:Lw],
                                            mb[:, c.Tw - c.W:c.Tw], OP.mult)
                    dbb_w = chpc("db", g).broadcast_to([128, c.W])
                    dbb_m = chpc("db", g).broadcast_to([128, c.Tout])
                    bscr = p2s.tile([128, c.W], BF16, name="bscr", tag="bscr")
                    nc.vector.tensor_tensor_scan(
                        bscr[:, ::-1], dbb_w, d1p[:, ::-1],
                        0.0, OP.mult, OP.add)
                    nc.vector.tensor_tensor_scan(
                        gb_t[g][:, ::-1], dbb_m, d1[:, 0:c.Tout][:, ::-1],
                        bscr[:, 0:1], OP.mult, OP.add)

                ABs = [None] * c.WCH
                BBs = [None] * c.WCH
                E2s = [None] * c.WCH

                # chunk-0 then chunk-1 reduction chains; the slot/mem
                # chains carry no ACT dependency and bridge the PE stream
                # while the entropy exps wait for the last phase-1 silu
                for g in range(c.HG):
                    red_fl(g, 0)
                for g in range(c.HG):
                    red_zg(g)

                def gatesA(w):
                    """Slot-softmax + memory-gate path for chunk w — no
                    dependency on the entropy exps, so this runs during the
                    ACT exp batch and feeds the memory matmuls early.  Also
                    drains pL/pM, freeing those banks for chunk w+1."""
                    E = p3.tile([128, c.CH], BF16, name="E", tag="E")
                    nc.scalar.activation(E[:], pL[w][:], AF.Exp,
                                         bias=bslot_t[:])
                    # reuse row 0 of the (now dead) slot-logit bank for Zs
                    pZs = pL[w][0:1, :]
                    nc.tensor.matmul(pZs, ones_t[:], E[:],
                                     start=True, stop=True)
                    mgs = row.tile([1, c.CH], F32, tag="mgs")
                    nc.scalar.activation(mgs[:], pM[w][:], AF.Exp,
                                         scale=-1.0, bias=scc(SC_NBMG))
                    mg1 = row.tile([1, c.CH], F32, tag="mg1")
                    nc.vector.tensor_scalar(mg1[:], mgs[:], 1.0, None, OP.add)
                    mgi = row.tile([1, c.CH], F32, tag="mgi")
                    nc.vector.reciprocal(mgi[:], mg1[:])
                    Zsr = row.tile([1, c.CH], F32, tag="Zsr")
                    nc.vector.reciprocal(Zsr[:], pZs)
                    s2 = row.tile([1, c.CH], BF16, tag="s2")
                    nc.vector.scalar_tensor_tensor(s2[:], mgi[:], scc(SC_F2),
                                                   Zsr[:], OP.mult, OP.mult)
                    S2B = pb1.tile([128, c.CH], BF16, name="S2B", tag="S2B")
                    nc.gpsimd.partition_broadcast(S2B[:], s2[:])
                    E2 = p3.tile([128, c.CH], BF16, name="E2", tag="E2")
                    nc.vector.tensor_tensor(E2[:], E[:], S2B[:], OP.mult)
                    E2s[w] = E2

                def gatesB(w):
                    """Entropy gate rows for chunk w (consume pZ/pG)."""
                    Zr = row.tile([1, c.CH], F32, tag="Zr")
                    nc.vector.reciprocal(Zr[:], pZ[w][:])
                    lnZ = row.tile([1, c.CH], F32, tag="lnZ")
                    nc.scalar.activation(lnZ[:], pZ[w][:], AF.Ln)
                    gz = row.tile([1, c.CH], F32, tag="gz")
                    nc.vector.tensor_tensor(gz[:], pG[w][:], Zr[:], OP.mult)
                    ent = row.tile([1, c.CH], F32, tag="ent")
                    nc.vector.tensor_tensor(ent[:], lnZ[:], gz[:], OP.subtract)
                    sg = row.tile([1, c.CH], F32, tag="sg")
                    nc.scalar.activation(sg[:], ent[:], AF.Exp,
                                         scale=scc(SC_NSW), bias=scc(SC_NSB))
                    sg1 = row.tile([1, c.CH], F32, tag="sg1")
                    nc.vector.tensor_scalar(sg1[:], sg[:], 1.0, None, OP.add)
                    gate = row.tile([1, c.CH], F32, tag="gate")
                    nc.vector.reciprocal(gate[:], sg1[:])
                    A = row.tile([1, c.CH], BF16, tag="A")
                    nc.vector.tensor_scalar(A[:], gate[:], scc(SC_F1),
                                            scc(SC_F0), OP.mult, OP.add)
                    Br = row.tile([1, c.CH], BF16, tag="B")
                    nc.vector.tensor_scalar(Br[:], gate[:], scc(SC_NF1), None,
                                            OP.mult)
                    AB = pb1.tile([128, c.CH], BF16, name="AB", tag="AB")
                    nc.gpsimd.partition_broadcast(AB[:], A[:])
                    BB = pb1.tile([128, c.CH], BF16, name="BB", tag="BB")
                    nc.gpsimd.partition_broadcast(BB[:], Br[:])
                    ABs[w], BBs[w] = AB, BB

                def fuse_out(w):
                    sl = slice(w * c.CH, (w + 1) * c.CH)
                    AB, BB, E2 = ABs[w], BBs[w], E2s[w]
                    # fusion -> weighted (scan layout); the memory-read
                    # contribution is folded into the out-proj via
                    # Wmo = mem_bank @ W_out (host-precomputed).
                    w_t = []
                    for g in range(c.HG):
                        t1 = p3.tile([128, c.CH], BF16, tag="t1")
                        nc.vector.tensor_tensor(t1[:], f_t[g][:, sl], AB[:],
                                                OP.mult)
                        t2 = p3.tile([128, c.CH], BF16, tag="t2")
                        if g % 2 == 0:
                            nc.gpsimd.tensor_tensor(t2[:], gb_t[g][:, sl],
                                                    BB[:], OP.mult)
                        else:
                            nc.vector.tensor_tensor(t2[:], gb_t[g][:, sl],
                                                    BB[:], OP.mult)
                        wt = wpool.tile([128, c.CH], BF16, name=f"w{g}",
                                        tag=f"w{g}")
                        nc.vector.tensor_tensor(wt[:], t1[:], t2[:], OP.add)
                        w_t.append(wt)

                    # out-proj from resident weights (+ memory term),
                    # k-outer so each wt[k] feeds the PE as soon as the
                    # fusion for group k lands
                    HM = c.OG // 2
                    for mh in range(2):
                        po_tags = ["pZ0", "pG0", "pZ1", "pG1"]
                        pos = [pst[0].tile([128, c.CH], F32, name=f"po{m}",
                                           tag=po_tags[i])
                               for i, m in enumerate(
                                   range(mh * HM, (mh + 1) * HM))]
                        for i, m in enumerate(range(mh * HM, (mh + 1) * HM)):
                            nc.tensor.matmul(
                                pos[i][:], wmo_t[:, m * 128:(m + 1) * 128],
                                E2[:], start=True, stop=False)
                        for k in range(c.HG):
                            for i, m in enumerate(range(mh * HM,
                                                        (mh + 1) * HM)):
                                nc.tensor.matmul(
                                    pos[i][:],
                                    wo_t[:, k * c.O + m * 128:
                                         k * c.O + (m + 1) * 128],
                                    w_t[k][:],
                                    start=False, stop=(k == c.HG - 1))
                        for i, m in enumerate(range(mh * HM, (mh + 1) * HM)):
                            ob = p3.tile([128, c.CH], F32, tag="ob")
                            if i % 2 == 0:
                                nc.scalar.activation(ob[:], pos[i][:],
                                                     AF.Identity,
                                                     bias=bout_t[:, m:m + 1])
                            else:
                                nc.vector.tensor_scalar(
                                    ob[:], pos[i][:], bout_t[:, m:m + 1],
                                    None, OP.add)
                            nc.sync.dma_start(
                                out_d[m * 128:(m + 1) * 128, sl], ob[:])

                # pipeline: chunk 1's PE reduction chains are emitted right
                # after chunk 0's gates so they fill the PE idle while the
                # DVE/ACT run chunk 0's rows and fusion.
                gatesA(0)
                for g in range(c.HG):
                    red_fl(g, 1)
                gatesA(1)
                for g in range(c.HG):
                    red_zg(g, 0)
                gatesB(0)
                for g in range(c.HG):
                    red_zg(g, 1)
                gatesB(1)
                fuse_out(0)
                fuse_out(1)

    nc.compile()
    return nc


_PROG_CACHE = {}


def _get_prog(cfg: Cfg):
    key = (cfg.DI, cfg.H, cfg.O, cfg.S, cfg.T, cfg.W, cfg.CH, cfg.sim_acts)
    if key not in _PROG_CACHE:
        _PROG_CACHE[key] = build_program(cfg)
    return _PROG_CACHE[key]


def make_in_maps(cfg, x, W_in, b_in, dt_bias_fwd, conv_k, decay_fwd, decay_bwd,
                 memory, mem_decay, W_mem_gate, b_mem_gate, W_slot, b_slot,
                 W_slot_bwd, b_slot_bwd, fusion_weight, scaler_w, scaler_b,
                 W_out, b_out):
    c = cfg
    x = np.asarray(x)
    B, T, DI = x.shape
    f32 = np.float32
    bf16 = ml_dtypes.bfloat16

    def sig(v):
        return 1.0 / (1.0 + np.exp(-np.asarray(v, np.float64)))

    def col(v):  # [H] -> [128, HG] column blocks
        return np.ascontiguousarray(np.asarray(v, f32).reshape(c.HG, 128).T)

    df = sig(decay_fwd)
    db = sig(decay_bwd)
    omdf = 1.0 - df
    k0 = np.asarray(conv_k, np.float64)[:, 0]
    k1 = np.asarray(conv_k, np.float64)[:, 1]
    prm = np.concatenate([
        col(k0 / k1),
        col(df), col(db),
        col(1.0 - db), col(omdf),
        col(np.asarray(b_in)[c.H:] + np.asarray(dt_bias_fwd)),
        col(np.asarray(b_in)[:c.H] * k1),
        np.ascontiguousarray(np.asarray(b_out, f32).reshape(c.OG, 128).T),
        np.asarray(b_slot_bwd, f32).reshape(128, 1),
    ], axis=1).astype(f32)
    scv = np.zeros((1, 8), f32)
    scv[0, SC_F1] = fusion_weight[1]
    scv[0, SC_F0] = fusion_weight[0]
    scv[0, SC_NF1] = -fusion_weight[1]
    scv[0, SC_F2] = fusion_weight[2]
    scv[0, SC_NSW] = -scaler_w[0]
    scv[0, SC_NSB] = -scaler_b[0]
    scv[0, SC_NBMG] = -b_mem_gate[0]
    mem_bank = np.asarray(memory, np.float64) * sig(mem_decay)[:, None]
    wmo = (mem_bank @ np.asarray(W_out, np.float64)).astype(bf16)
    wslot_s = np.asarray(W_slot_bwd, np.float64).astype(bf16)
    wmg_s = np.asarray(W_mem_gate, np.float64).astype(bf16)
    # fold k1 (conv center tap) into the 'a' half of the in-projection
    w_in_s = np.asarray(W_in, np.float64).copy()
    w_in_s[:, :c.H] *= k1[None, :]

    shared = {
        "w_in": np.ascontiguousarray(w_in_s).astype(bf16),
        "w_out": np.ascontiguousarray(np.asarray(W_out)).astype(bf16),
        "w_slot": np.ascontiguousarray(wslot_s),
        "w_mg": np.ascontiguousarray(wmg_s),
        "wmo": wmo,
        "prm": prm, "sc": scv,
    }
    in_maps = []
    for core in range(8):
        b, j = divmod(core, 2)
        start = j * c.Tout - c.W
        gs, ge = max(0, start), min(T, start + c.Tw)
        xt = np.zeros((c.DI, c.Tw), bf16)
        xt[:, gs - start:ge - start] = x[b, gs:ge, :].T.astype(bf16)
        mask = np.zeros((1, c.Tw), bf16)
        mask[0, gs - start:ge - start] = 1.0
        m = dict(shared)
        m["xt"] = xt
        m["mask"] = mask
        in_maps.append(m)
    return in_maps


def run(cfg, inputs, trace=False, tmpdir=None):
    nc = _get_prog(cfg)
    in_maps = make_in_maps(cfg, **inputs)
    res = run_bass_kernel_spmd(nc, in_maps, core_ids=list(range(8)),
                               trace=trace, tmpdir=tmpdir)
    B, T = np.asarray(inputs["x"]).shape[0], np.asarray(inputs["x"]).shape[1]
    out = np.empty((B, T, cfg.O), np.float32)
    for core in range(8):
        b, j = divmod(core, 2)
        out[b, j * cfg.Tout:(j + 1) * cfg.Tout, :] = res.results[core]["outT"].T
    return out, res


def kernel(**inputs):
    cfg = Cfg()
    out, _ = run(cfg, inputs)
    return out


# revision 71
# speedup vs baseline: 1.0204x; 1.0204x over previous
"""Trainium2 Bass kernel for the HNX scatter-memory block.

Sharding: 8 cores = (batch b in 0..3) x (sequence half j in 0..1).
Each core processes its 1024-token window plus W warmup tokens on each
side (zero-padded + masked at sequence edges), so both the forward and
backward EMA scans converge to near the exact state before the window
starts (truncation error ~ sigmoid(decay)^W ~ 2e-3 at W=32).  No
inter-core communication.

On-chip layout is "scan layout": channels on partitions, time along the
free dimension.  Key optimizations over a straightforward port:

- All matmul operands and elementwise intermediates are bf16 (fp32 PSUM
  accumulation, fp32 per-channel scalars): 1 cycle/row on the PE and
  the 2x/4x DVE perf modes.
- The conv center tap k1 is folded into the in-proj weights on the
  host, so the causal conv is y_t = (k0/k1)*x1_{t-1} + x1_t (one 4x
  tensor_scalar + one gpsimd add).
- The memory-read contribution is folded into the out-projection via
  Wmo = mem_bank @ W_out (host-precomputed): W_out^T(memb^T E2) =
  Wmo^T E2 joins the out-proj PSUM accumulation chains directly.
- Phase-1 elementwise runs per 512-token chunk, pipelined with the
  in-proj matmul chains; the entropy/slot/mem reduction chains and the
  second window chunk's work are emission-ordered to keep the PE busy
  through the gate/fusion tail.
- ACT activation-table thrash is avoided by ordering all exps after
  the last phase-1 silu (explicit dep) and keeping the gate sigmoids
  in exp/reciprocal form (same ACT table as Ln/Exp/Identity).
- DMA issue costs ~650ns of sequencer time per dma_start, so inputs
  are consolidated (one packed param tensor, one xt tile, two-group
  weight loads) and issued from otherwise-idle engine queues.
"""

import numpy as np
from contextlib import ExitStack

import ml_dtypes
import concourse.bacc as bacc
import concourse.tile as tile
from concourse import mybir
from concourse.bass_utils import run_bass_kernel_spmd

F32 = mybir.dt.float32
BF16 = mybir.dt.bfloat16
AF = mybir.ActivationFunctionType
OP = mybir.AluOpType


class Cfg:
    def __init__(self, DI=1024, H=1024, O=1024, S=128, T=2048, W=64, CH=512,
                 sim_acts=False):
        self.DI, self.H, self.O, self.S, self.T, self.W, self.CH = DI, H, O, S, T, W, CH
        self.Tout = T // 2            # tokens per core window
        self.Tw = self.Tout + 2 * W   # work tokens per core
        self.KG = DI // 128           # input k-tiles
        self.HG = H // 128            # hidden channel groups
        self.OG = O // 128            # output channel groups
        self.NCH = self.Tw // CH      # phase-1 chunks
        self.WCH = self.Tout // CH    # phase-3 (window) chunks
        self.sim_acts = sim_acts
        assert self.S == 128 and self.Tout % CH == 0
        assert CH <= 512 and self.W <= CH


# chp column layout: per-channel params, one column per (param, group)
CHP_NAMES = ["kr", "df", "db", "omdb", "omdf", "sbias", "ba"]
SC_F1, SC_F0, SC_NF1, SC_F2, SC_NSW, SC_NSB, SC_NBMG = range(7)


def build_program(cfg: Cfg):
    c = cfg
    nc = bacc.Bacc("TRN2", target_bir_lowering=False, debug=False,
                   enable_asserts=False)

    NPRM = len(CHP_NAMES) * c.HG + c.OG + 1
    xt = nc.dram_tensor("xt", [c.DI, c.Tw], BF16, kind="ExternalInput").ap()
    w_in = nc.dram_tensor("w_in", [c.DI, 2 * c.H], BF16, kind="ExternalInput").ap()
    w_out = nc.dram_tensor("w_out", [c.H, c.O], BF16, kind="ExternalInput").ap()
    w_slot = nc.dram_tensor("w_slot", [c.H, c.S], BF16, kind="ExternalInput").ap()
    w_mg = nc.dram_tensor("w_mg", [c.H, 1], BF16, kind="ExternalInput").ap()
    wmo_d = nc.dram_tensor("wmo", [c.S, c.O], BF16, kind="ExternalInput").ap()
    prm = nc.dram_tensor("prm", [128, NPRM], F32, kind="ExternalInput").ap()
    sc = nc.dram_tensor("sc", [1, 8], F32, kind="ExternalInput").ap()
    mask_d = nc.dram_tensor("mask", [1, c.Tw], BF16, kind="ExternalInput").ap()
    out_d = nc.dram_tensor("outT", [c.O, c.Tout], F32, kind="ExternalOutput").ap()

    with tile.TileContext(nc) as tc:
        with ExitStack() as top:
            consts = top.enter_context(tc.tile_pool(name="consts", bufs=1))
            # critical-path first: each dma_start costs ~650ns of issue time
            # on its issuing engine's sequencer, so the head transfers are
            # consolidated and spread across the idle engines.
            prm_t = consts.tile([128, NPRM], F32)
            nc.sync.dma_start(prm_t[:], prm[:])
            sc_t = consts.tile([1, 8], F32)
            nc.sync.dma_start(sc_t[:], sc[:])
            NCH0 = len(CHP_NAMES) * c.HG
            bout_t = prm_t[:, NCH0:NCH0 + c.OG]
            bslot_t = prm_t[:, NCH0 + c.OG:NCH0 + c.OG + 1]

            # x in scan layout, all k-tiles in one tile [128, k*Tw]
            xt_pool = top.enter_context(tc.tile_pool(name="xt", bufs=1))
            xta = xt_pool.tile([128, c.KG * c.Tw], BF16)
            xta_v = xta[:].rearrange("p (k t) -> p k t", t=c.Tw)
            xt_v = xt.rearrange("(k p) t -> p k t", p=128)
            KH = c.KG // 2
            nc.gpsimd.dma_start(xta_v[:, 0:KH, 0:c.CH], xt_v[:, 0:KH, 0:c.CH])
            nc.gpsimd.dma_start(xta_v[:, KH:c.KG, 0:c.CH],
                                xt_v[:, KH:c.KG, 0:c.CH])

            def xts(k, sl):
                return xta[:, k * c.Tw:(k + 1) * c.Tw][:, sl]

            win = top.enter_context(tc.tile_pool(name="win", bufs=2))

            WG = 2   # weight groups per DMA

            def load_w(g, eng_a=None, eng_d=None):
                """Load in-proj weights for groups [g, g+WG).  The pdt
                chain runs first, so wd transfers first."""
                gw = 128 * WG
                wd = win.tile([128, gw * c.KG], BF16, name="wd", tag="wd")
                (eng_d or nc.sync).dma_start(
                    wd[:].rearrange("p (k m) -> p k m", m=gw),
                    w_in[:, c.H + g * 128:c.H + g * 128 + gw]
                    .rearrange("(k p) m -> p k m", p=128))
                wa = win.tile([128, gw * c.KG], BF16, name="wa", tag="wa")
                (eng_a or nc.sync).dma_start(
                    wa[:].rearrange("p (k m) -> p k m", m=gw),
                    w_in[:, g * 128:g * 128 + gw]
                    .rearrange("(k p) m -> p k m", p=128))
                return wa, wd

            w_pre = load_w(0, eng_a=nc.scalar, eng_d=nc.scalar)
            ones_t = consts.tile([128, 1], BF16)
            nc.vector.memset(ones_t[:], 1.0)
            mb = consts.tile([128, c.Tw], BF16)
            nc.sync.dma_start(mb[:], mask_d.broadcast_to([128, c.Tw]))
            wsa = consts.tile([128, c.HG * c.S], BF16)
            nc.sync.dma_start(
                wsa[:].rearrange("p (k s) -> p k s", s=c.S),
                w_slot.rearrange("(k p) s -> p k s", p=128))
            wslot_t = [wsa[:, k * c.S:(k + 1) * c.S] for k in range(c.HG)]
            wmga = consts.tile([128, c.HG], BF16)
            nc.sync.dma_start(
                wmga[:].rearrange("p (k s) -> p k s", s=1),
                w_mg.rearrange("(k p) s -> p k s", p=128))
            wmg_t = [wmga[:, k:k + 1] for k in range(c.HG)]
            nc.sync.dma_start(xta_v[:, :, c.CH:c.Tw], xt_v[:, :, c.CH:c.Tw])
            wmo_t = consts.tile([128, c.O], BF16)
            # resident out-proj weights: [128, KG*OG*128] bf16, block (k,m)
            wo_t = consts.tile([128, c.HG * c.O], BF16)

            def chpc(name, g):
                i = CHP_NAMES.index(name) * c.HG + g
                return prm_t[:, i:i + 1]

            def scc(i):
                return sc_t[0:1, i:i + 1]

            fpool = top.enter_context(tc.tile_pool(name="f", bufs=1))
            f_t = [fpool.tile([128, c.Tw - c.W], BF16, name=f"f{g}", tag=f"f{g}")
                   for g in range(c.HG)]
            gpool = top.enter_context(tc.tile_pool(name="gb", bufs=1))
            gb_t = [gpool.tile([128, c.Tout], BF16, name=f"gb{g}", tag=f"gb{g}")
                    for g in range(c.HG)]

            pch = top.enter_context(tc.tile_pool(name="pch", bufs=3))
            scr = top.enter_context(tc.tile_pool(name="scr", bufs=2))
            p2s = top.enter_context(tc.tile_pool(name="p2s", bufs=2))
            p3 = top.enter_context(tc.tile_pool(name="p3", bufs=3))

            # phase-3 accumulators (PSUM): one bank per (pZ,pG,pM,pL),
            # single-buffered and reused across the two window chunks.
            pst = [None]   # tail-scope PSUM pool, created after phase 1
            pZ = [None] * c.WCH
            pG = [None] * c.WCH
            pM = [None] * c.WCH
            pL = [None] * c.WCH

            last_silu = [None]

            def red_fl(g, w):
                """Slot-logit + mem-gate chains: depend only on f_t[g], so
                chunk 0's run interleaved with phase 1 on the PE."""
                sl = slice(w * c.CH, (w + 1) * c.CH)
                if g == 0:
                    pM[w] = pst[0].tile([1, c.CH], F32, name=f"pM{w}",
                                        tag=f"pM{w}")
                    pL[w] = pst[0].tile([128, c.CH], F32, name=f"pL{w}",
                                        tag=f"pL{w}")
                st, sp = (g == 0), (g == c.HG - 1)
                nc.tensor.matmul(pM[w][:], wmg_t[g][:],
                                 f_t[g][:, sl], start=st, stop=sp)
                nc.tensor.matmul(pL[w][:], wslot_t[g][:],
                                 f_t[g][:, sl], start=st, stop=sp)

            def red_zg(g, w):
                """Entropy chains (need the exp of f).  The exps are
                ordered after the last phase-1 silu (ACT-table batching)."""
                sl = slice(w * c.CH, (w + 1) * c.CH)
                if g == 0:
                    pZ[w] = pst[0].tile([1, c.CH], F32, name=f"pZ{w}",
                                        tag=f"pZ{w}")
                    pG[w] = pst[0].tile([1, c.CH], F32, name=f"pG{w}",
                                        tag=f"pG{w}")
                st, sp = (g == 0), (g == c.HG - 1)
                pt = p3.tile([128, c.CH], BF16, name="pt", tag="p")
                pt_inst = nc.scalar.activation(pt[:], f_t[g][:, sl], AF.Exp)
                if last_silu[0] is not None:
                    tile.add_dep_helper(pt_inst.ins, last_silu[0].ins,
                                        sync=False,
                                        reason="act-table batching")
                pft = p3.tile([128, c.CH], BF16, name="pft", tag="pf")
                nc.vector.tensor_tensor(pft[:], f_t[g][:, sl],
                                        pt[:], OP.mult)
                nc.tensor.matmul(pZ[w][:], ones_t[:], pt[:],
                                 start=st, stop=sp)
                nc.tensor.matmul(pG[w][:], ones_t[:], pft[:],
                                 start=st, stop=sp)

            with ExitStack() as p1:
                ps1 = p1.enter_context(tc.tile_pool(name="ps1", bufs=2,
                                                    space="PSUM"))

                # phase-1 chunk edges (last chunk may be short)
                edges = list(range(0, c.Tw, c.CH)) + [c.Tw]

                for g in range(c.HG):
                    if g % WG == 0:
                        wa2, wd2 = w_pre
                        if g + WG < c.HG:
                            w_pre = load_w(g + WG)
                    gg = g % WG
                    gw = 128 * WG

                    def wslice(wt, k):
                        return wt[:, k * gw + gg * 128:k * gw + (gg + 1) * 128]

                    x1 = pch.tile([128, c.Tw], BF16, tag="x1")
                    tmp = pch.tile([128, c.Tw], BF16, tag="tc")
                    ypre = pch.tile([128, c.Tw], BF16, tag="tb")
                    ysl = pch.tile([128, c.Tw], BF16, tag="ysl")
                    u = pch.tile([128, c.Tw], BF16, tag="ta")
                    fscr = scr.tile([128, c.W], BF16, tag="fscr")
                    for n in range(len(edges) - 1):
                        sl = slice(edges[n], edges[n + 1])
                        cw = edges[n + 1] - edges[n]
                        pa = ps1.tile([128, cw], F32, tag="pa")
                        pdt = ps1.tile([128, cw], F32, tag="pdt")
                        for k in range(c.KG):
                            nc.tensor.matmul(
                                pdt[:], wslice(wd2, k),
                                xts(k, sl),
                                start=(k == 0), stop=(k == c.KG - 1))
                        for k in range(c.KG):
                            nc.tensor.matmul(
                                pa[:], wslice(wa2, k),
                                xts(k, sl),
                                start=(k == 0), stop=(k == c.KG - 1))
                        sdt = scr.tile([128, cw], BF16, tag="sdt")
                        if c.sim_acts:
                            # CoreSim has no Silu: silu(x) = x * sigmoid(x)
                            nc.scalar.activation(sdt[:], pdt[:], AF.Sigmoid,
                                                 bias=chpc("sbias", g))
                            nc.vector.scalar_tensor_tensor(
                                sdt[:], pdt[:], chpc("sbias", g), sdt[:],
                                OP.add, OP.mult)
                        else:
                            nc.scalar.activation(sdt[:], pdt[:], AF.Silu,
                                                 bias=chpc("sbias", g))
                        if n == 0 or n == len(edges) - 2:
                            # zero the pad region so the causal conv and
                            # scans see zeros outside the true sequence
                            nc.vector.tensor_tensor(sdt[:], sdt[:],
                                                    mb[:, sl], OP.mult)
                        # x1 = (k1*a + k1*b_a) * silu(dt + sbias)
                        # (k1 is folded into W_in / b_a on the host)
                        nc.vector.scalar_tensor_tensor(
                            x1[:, sl], pa[:], chpc("ba", g), sdt[:],
                            OP.add, OP.mult)

                    # per-chunk conv + silu + scan pipeline, in a second
                    # chunk loop so all the sdt acts precede the (slower)
                    # ysl chain on ACT — otherwise ysl head-of-line blocks
                    # the sdt that frees the next PSUM bank.
                    for n in range(len(edges) - 1):
                        s, e = edges[n], edges[n + 1]
                        if n == 0:
                            nc.gpsimd.memset(tmp[:, 0:1], 0.0)
                            nc.vector.tensor_scalar(
                                tmp[:, 1:e], x1[:, 0:e - 1],
                                chpc("kr", g), None, OP.mult)
                        else:
                            nc.vector.tensor_scalar(
                                tmp[:, s:e], x1[:, s - 1:e - 1],
                                chpc("kr", g), None, OP.mult)
                        if g >= c.HG - 2:
                            # last groups sit on the critical path into the
                            # reductions: use the faster engine
                            nc.vector.tensor_tensor(
                                ypre[:, s:e], tmp[:, s:e], x1[:, s:e], OP.add)
                        else:
                            nc.gpsimd.tensor_tensor(
                                ypre[:, s:e], tmp[:, s:e], x1[:, s:e], OP.add)
                        if c.sim_acts:
                            last_silu[0] = nc.scalar.activation(
                                ysl[:, s:e], ypre[:, s:e], AF.Sigmoid)
                            nc.vector.tensor_tensor(
                                ysl[:, s:e], ypre[:, s:e], ysl[:, s:e],
                                OP.mult)
                        else:
                            last_silu[0] = nc.scalar.activation(
                                ysl[:, s:e], ypre[:, s:e], AF.Silu)
                        nc.vector.tensor_scalar(u[:, s:e], ysl[:, s:e],
                                                chpc("omdf", g), None, OP.mult)

                        # fwd scan f = df*f + (1-df)*y, chained across
                        # chunks; first W tokens go to discard scratch
                        dfb = chpc("df", g)
                        if n == 0:
                            nc.vector.tensor_tensor_scan(
                                fscr[:], dfb.broadcast_to([128, c.W]),
                                u[:, 0:c.W], 0.0, OP.mult, OP.add)
                            nc.vector.tensor_tensor_scan(
                                f_t[g][:, 0:e - c.W],
                                dfb.broadcast_to([128, e - c.W]),
                                u[:, c.W:e], fscr[:, c.W - 1:c.W],
                                OP.mult, OP.add)
                        else:
                            nc.vector.tensor_tensor_scan(
                                f_t[g][:, s - c.W:e - c.W],
                                dfb.broadcast_to([128, e - s]),
                                u[:, s:e],
                                f_t[g][:, s - c.W - 1:s - c.W],
                                OP.mult, OP.add)

                # tail-only weights: emitted late so their DMAs don't
                # compete with the phase-1 critical path.
                nc.sync.dma_start(wmo_t[:], wmo_d[:])
                for k in range(c.HG):
                    nc.sync.dma_start(wo_t[:, k * c.O:(k + 1) * c.O],
                                      w_out[k * 128:(k + 1) * 128, :])


            # ------------- phase 3 tail: gates, memory, fusion, out ------
            with ExitStack() as p2:
                pb1 = p2.enter_context(tc.tile_pool(name="pb1", bufs=2))
                wpool = p2.enter_context(tc.tile_pool(name="wp", bufs=1))
                row = p2.enter_context(tc.tile_pool(name="row", bufs=1))
                # one bank per accumulator (both chunks fully
                # independent); the out-proj chains reuse four of these
                # banks by tag after the rows drain them.
                pst[0] = p2.enter_context(tc.tile_pool(name="pst", bufs=1,
                                                       space="PSUM"))

                # ---- phase 2: bwd scans (gb only feeds late fusion) ----
                for g in range(c.HG):
                    Lw = c.Tw - c.W
                    d1 = p2s.tile([128, Lw], BF16, name="d1", tag="d1")
                    nc.vector.tensor_scalar(d1[:], f_t[g][:], chpc("omdb", g),
                                            None, OP.mult)
                    # mask only matters in the right-pad = bwd warmup region
                    d1p = p2s.tile([128, c.W], BF16, name="d1p", tag="d1p")
                    nc.vector.tensor_tensor(d1p[:], d1[:, c.Tout:Lw],
                                            mb[:, c.Tw - c.W:c.Tw], OP.mult)
                    dbb_w = chpc("db", g).broadcast_to([128, c.W])
                    dbb_m = chpc("db", g).broadcast_to([128, c.Tout])
                    bscr = p2s.tile([128, c.W], BF16, name="bscr", tag="bscr")
                    nc.vector.tensor_tensor_scan(
                        bscr[:, ::-1], dbb_w, d1p[:, ::-1],
                        0.0, OP.mult, OP.add)
                    nc.vector.tensor_tensor_scan(
                        gb_t[g][:, ::-1], dbb_m, d1[:, 0:c.Tout][:, ::-1],
                        bscr[:, 0:1], OP.mult, OP.add)

                ABs = [None] * c.WCH
                BBs = [None] * c.WCH
                E2s = [None] * c.WCH

                # chunk-0 then chunk-1 reduction chains; the slot/mem
                # chains carry no ACT dependency and bridge the PE stream
                # while the entropy exps wait for the last phase-1 silu
                for g in range(c.HG):
                    red_fl(g, 0)
                for g in range(c.HG):
                    red_zg(g, 0)

                def gatesA(w):
                    """Slot-softmax + memory-gate path for chunk w — no
                    dependency on the entropy exps, so this runs during the
                    ACT exp batch and feeds the memory matmuls early.  Also
                    drains pL/pM, freeing those banks for chunk w+1."""
                    E = p3.tile([128, c.CH], BF16, name="E", tag="E")
                    nc.scalar.activation(E[:], pL[w][:], AF.Exp,
                                         bias=bslot_t[:])
                    # reuse row 0 of the (now dead) slot-logit bank for Zs
                    pZs = pL[w][0:1, :]
                    nc.tensor.matmul(pZs, ones_t[:], E[:],
                                     start=True, stop=True)
                    mgs = row.tile([1, c.CH], F32, tag="mgs")
                    nc.scalar.activation(mgs[:], pM[w][:], AF.Exp,
                                         scale=-1.0, bias=scc(SC_NBMG))
                    mg1 = row.tile([1, c.CH], F32, tag="mg1")
                    nc.vector.tensor_scalar(mg1[:], mgs[:], 1.0, None, OP.add)
                    mgi = row.tile([1, c.CH], F32, tag="mgi")
                    nc.vector.reciprocal(mgi[:], mg1[:])
                    Zsr = row.tile([1, c.CH], F32, tag="Zsr")
                    nc.vector.reciprocal(Zsr[:], pZs)
                    s2 = row.tile([1, c.CH], BF16, tag="s2")
                    nc.vector.scalar_tensor_tensor(s2[:], mgi[:], scc(SC_F2),
                                                   Zsr[:], OP.mult, OP.mult)
                    S2B = pb1.tile([128, c.CH], BF16, name="S2B", tag="S2B")
                    nc.gpsimd.partition_broadcast(S2B[:], s2[:])
                    E2 = p3.tile([128, c.CH], BF16, name="E2", tag="E2")
                    nc.vector.tensor_tensor(E2[:], E[:], S2B[:], OP.mult)
                    E2s[w] = E2

                def gatesB(w):
                    """Entropy gate rows for chunk w (consume pZ/pG)."""
                    Zr = row.tile([1, c.CH], F32, tag="Zr")
                    nc.vector.reciprocal(Zr[:], pZ[w][:])
                    lnZ = row.tile([1, c.CH], F32, tag="lnZ")
                    nc.scalar.activation(lnZ[:], pZ[w][:], AF.Ln)
                    gz = row.tile([1, c.CH], F32, tag="gz")
                    nc.vector.tensor_tensor(gz[:], pG[w][:], Zr[:], OP.mult)
                    ent = row.tile([1, c.CH], F32, tag="ent")
                    nc.vector.tensor_tensor(ent[:], lnZ[:], gz[:], OP.subtract)
                    sg = row.tile([1, c.CH], F32, tag="sg")
                    nc.scalar.activation(sg[:], ent[:], AF.Exp,
                                         scale=scc(SC_NSW), bias=scc(SC_NSB))
                    sg1 = row.tile([1, c.CH], F32, tag="sg1")
                    nc.vector.tensor_scalar(sg1[:], sg[:], 1.0, None, OP.add)
                    gate = row.tile([1, c.CH], F32, tag="gate")
                    nc.vector.reciprocal(gate[:], sg1[:])
                    A = row.tile([1, c.CH], BF16, tag="A")
                    nc.vector.tensor_scalar(A[:], gate[:], scc(SC_F1),
                                            scc(SC_F0), OP.mult, OP.add)
                    Br = row.tile([1, c.CH], BF16, tag="B")
                    nc.vector.tensor_scalar(Br[:], gate[:], scc(SC_NF1), None,
                                            OP.mult)
                    AB = pb1.tile([128, c.CH], BF16, name="AB", tag="AB")
                    nc.gpsimd.partition_broadcast(AB[:], A[:])
                    BB = pb1.tile([128, c.CH], BF16, name="BB", tag="BB")
                    nc.gpsimd.partition_broadcast(BB[:], Br[:])
                    ABs[w], BBs[w] = AB, BB

                def fuse_out(w):
                    sl = slice(w * c.CH, (w + 1) * c.CH)
                    AB, BB, E2 = ABs[w], BBs[w], E2s[w]
                    # fusion -> weighted (scan layout); the memory-read
                    # contribution is folded into the out-proj via
                    # Wmo = mem_bank @ W_out (host-precomputed).
                    w_t = []
                    for g in range(c.HG):
                        t1 = p3.tile([128, c.CH], BF16, tag="t1")
                        nc.vector.tensor_tensor(t1[:], f_t[g][:, sl], AB[:],
                                                OP.mult)
                        t2 = p3.tile([128, c.CH], BF16, tag="t2")
                        if g % 2 == 0:
                            nc.gpsimd.tensor_tensor(t2[:], gb_t[g][:, sl],
                                                    BB[:], OP.mult)
                        else:
                            nc.vector.tensor_tensor(t2[:], gb_t[g][:, sl],
                                                    BB[:], OP.mult)
                        wt = wpool.tile([128, c.CH], BF16, name=f"w{g}",
                                        tag=f"w{g}")
                        nc.vector.tensor_tensor(wt[:], t1[:], t2[:], OP.add)
                        w_t.append(wt)

                    # out-proj from resident weights (+ memory term),
                    # k-outer so each wt[k] feeds the PE as soon as the
                    # fusion for group k lands
                    HM = c.OG // 2
                    for mh in range(2):
                        po_tags = ["pZ0", "pG0", "pZ1", "pG1"]
                        pos = [pst[0].tile([128, c.CH], F32, name=f"po{m}",
                                           tag=po_tags[i])
                               for i, m in enumerate(
                                   range(mh * HM, (mh + 1) * HM))]
                        for i, m in enumerate(range(mh * HM, (mh + 1) * HM)):
                            nc.tensor.matmul(
                                pos[i][:], wmo_t[:, m * 128:(m + 1) * 128],
                                E2[:], start=True, stop=False)
                        for k in range(c.HG):
                            for i, m in enumerate(range(mh * HM,
                                                        (mh + 1) * HM)):
                                nc.tensor.matmul(
                                    pos[i][:],
                                    wo_t[:, k * c.O + m * 128:
                                         k * c.O + (m + 1) * 128],
                                    w_t[k][:],
                                    start=False, stop=(k == c.HG - 1))
                        for i, m in enumerate(range(mh * HM, (mh + 1) * HM)):
                            ob = p3.tile([128, c.CH], F32, tag="ob")
                            if i % 2 == 0:
                                nc.scalar.activation(ob[:], pos[i][:],
                                                     AF.Identity,
                                                     bias=bout_t[:, m:m + 1])
                            else:
                                nc.vector.tensor_scalar(
                                    ob[:], pos[i][:], bout_t[:, m:m + 1],
                                    None, OP.add)
                            nc.sync.dma_start(
                                out_d[m * 128:(m + 1) * 128, sl], ob[:])

                # pipeline: chunk 1's PE reduction chains are emitted right
                # after chunk 0's gates so they fill the PE idle while the
                # DVE/ACT run chunk 0's rows and fusion.
                gatesA(0)
                for g in range(c.HG):
                    red_fl(g, 1)
                gatesA(1)
                for g in range(c.HG):
                    red_zg(g, 0)
                gatesB(0)
                for g in range(c.HG):
                    red_zg(g, 1)
                gatesB(1)
                fuse_out(0)
                fuse_out(1)

    nc.compile()
    return nc


_PROG_CACHE = {}


def _get_prog(cfg: Cfg):
    key = (cfg.DI, cfg.H, cfg.O, cfg.S, cfg.T, cfg.W, cfg.CH, cfg.sim_acts)
    if key not in _PROG_CACHE:
        _PROG_CACHE[key] = build_program(cfg)
    return _PROG_CACHE[key]


def make_in_maps(cfg, x, W_in, b_in, dt_bias_fwd, conv_k, decay_fwd, decay_bwd,
                 memory, mem_decay, W_mem_gate, b_mem_gate, W_slot, b_slot,
                 W_slot_bwd, b_slot_bwd, fusion_weight, scaler_w, scaler_b,
                 W_out, b_out):
    c = cfg
    x = np.asarray(x)
    B, T, DI = x.shape
    f32 = np.float32
    bf16 = ml_dtypes.bfloat16

    def sig(v):
        return 1.0 / (1.0 + np.exp(-np.asarray(v, np.float64)))

    def col(v):  # [H] -> [128, HG] column blocks
        return np.ascontiguousarray(np.asarray(v, f32).reshape(c.HG, 128).T)

    df = sig(decay_fwd)
    db = sig(decay_bwd)
    omdf = 1.0 - df
    k0 = np.asarray(conv_k, np.float64)[:, 0]
    k1 = np.asarray(conv_k, np.float64)[:, 1]
    prm = np.concatenate([
        col(k0 / k1),
        col(df), col(db),
        col(1.0 - db), col(omdf),
        col(np.asarray(b_in)[c.H:] + np.asarray(dt_bias_fwd)),
        col(np.asarray(b_in)[:c.H] * k1),
        np.ascontiguousarray(np.asarray(b_out, f32).reshape(c.OG, 128).T),
        np.asarray(b_slot_bwd, f32).reshape(128, 1),
    ], axis=1).astype(f32)
    scv = np.zeros((1, 8), f32)
    scv[0, SC_F1] = fusion_weight[1]
    scv[0, SC_F0] = fusion_weight[0]
    scv[0, SC_NF1] = -fusion_weight[1]
    scv[0, SC_F2] = fusion_weight[2]
    scv[0, SC_NSW] = -scaler_w[0]
    scv[0, SC_NSB] = -scaler_b[0]
    scv[0, SC_NBMG] = -b_mem_gate[0]
    mem_bank = np.asarray(memory, np.float64) * sig(mem_decay)[:, None]
    wmo = (mem_bank @ np.asarray(W_out, np.float64)).astype(bf16)
    wslot_s = np.asarray(W_slot_bwd, np.float64).astype(bf16)
    wmg_s = np.asarray(W_mem_gate, np.float64).astype(bf16)
    # fold k1 (conv center tap) into the 'a' half of the in-projection
    w_in_s = np.asarray(W_in, np.float64).copy()
    w_in_s[:, :c.H] *= k1[None, :]

    shared = {
        "w_in": np.ascontiguousarray(w_in_s).astype(bf16),
        "w_out": np.ascontiguousarray(np.asarray(W_out)).astype(bf16),
        "w_slot": np.ascontiguousarray(wslot_s),
        "w_mg": np.ascontiguousarray(wmg_s),
        "wmo": wmo,
        "prm": prm, "sc": scv,
    }
    in_maps = []
    for core in range(8):
        b, j = divmod(core, 2)
        start = j * c.Tout - c.W
        gs, ge = max(0, start), min(T, start + c.Tw)
        xt = np.zeros((c.DI, c.Tw), bf16)
        xt[:, gs - start:ge - start] = x[b, gs:ge, :].T.astype(bf16)
        mask = np.zeros((1, c.Tw), bf16)
        mask[0, gs - start:ge - start] = 1.0
        m = dict(shared)
        m["xt"] = xt
        m["mask"] = mask
        in_maps.append(m)
    return in_maps


def run(cfg, inputs, trace=False, tmpdir=None):
    nc = _get_prog(cfg)
    in_maps = make_in_maps(cfg, **inputs)
    res = run_bass_kernel_spmd(nc, in_maps, core_ids=list(range(8)),
                               trace=trace, tmpdir=tmpdir)
    B, T = np.asarray(inputs["x"]).shape[0], np.asarray(inputs["x"]).shape[1]
    out = np.empty((B, T, cfg.O), np.float32)
    for core in range(8):
        b, j = divmod(core, 2)
        out[b, j * cfg.Tout:(j + 1) * cfg.Tout, :] = res.results[core]["outT"].T
    return out, res


def kernel(**inputs):
    cfg = Cfg()
    out, _ = run(cfg, inputs)
    return out
